# revision 2
# baseline (speedup 1.0000x reference)
"""v13: v12 + progressive input chunks for the earliest PE start.

Stage-1 input arrives as four 512-col chunks right-to-left, matching
the rb-descending diagonal schedule: the PE starts after one 0.38MB
chunk and chunk arrival always outpaces tile consumption. The final
cross run is split into two single-tile output DMAs to shorten the
end-of-kernel drain.

fp32r internally rounds matmul operands to ~11-12 mantissa bits
(measured); fp16 carries 11 significant bits. Quantizing x to fp16 on
the host and running plain fp16 matmuls costs ~2x wider boundary noise
(absorbed by the exact host band refinement, BAND 0.03 -> 0.05) while
halving the 6MB input DMA -- the serial head before the PE can start.

- Diagonal-block tiles now cover exactly the upper wedge: row block rb
  spans cols [rb*128, 2048). Spans split into pieces of 512/384/256
  (never an isolated <256 remainder except the unavoidable 128-wide
  tip, since fp32r drops to 1/4 rate below 256 moving cols). Saves
  ~2.7k PE cycles/core and shrinks output bytes; host rebuilds the
  diagonal block with a triu+transpose.
- Stage-1 input arrives as cols [512:2048) then [0:512); diagonal rows
  are processed rb-descending so every tile has >=3us of data slack:
  no PE stalls (a stall also resets the PE's max p-state).
- y ops balanced between Act and DVE by accumulated columns (3:2.4
  effective rates); per-row-block run buffers leave in one DMA each.

Same math as v9/v10: fp32r matmuls, unique-tile cross partition, fp8
y=(G-thr)*256 output, host mirror + exact f64 band refinement.
"""

import sys

for _p in ("/opt/trn_rl_repo", "/root/.axon_site/_ro/trn_rl_repo"):
    if _p not in sys.path:
        sys.path.append(_p)

import numpy as np

B, C, N = 4, 384, 4096
HALF = N // 2
QTR = N // 4
KT = C // 128
NCORES = 2 * B
PPF_09 = 1.2815515655446004
EPS = 1e-12
RB = HALF // 128
YSCALE = 256.0
BAND = 0.05

_compiled_nc = None
_last_ctx = {}


def _diag_pieces(rb):
    """Split cols [rb*128, 2048) into moving pieces, all >=256 wide
    except an unavoidable final 128 tip."""
    j, end = rb * 128, HALF
    out = []
    while j < end:
        s = end - j
        if s == 128:
            out.append((j, 128))
            j = end
        elif s == 640:
            out.extend([(j, 384), (j + 384, 256)])
            j = end
        else:
            w = min(512, s)
            out.append((j, w))
            j += w
    return out


def _runs():
    """(i0, [(j0, w)...]) per row block: diag (rb descending), cross."""
    runs = []
    for rb in range(RB - 1, -1, -1):
        runs.append((rb * 128, _diag_pieces(rb)))
    for rb in range(8):
        runs.append((rb * 128, [(HALF, 512), (HALF + 512, 512)]))
    for rb in range(8, 15):
        runs.append((rb * 128, [(3 * QTR, 512), (3 * QTR + 512, 512)]))
    runs.append((15 * 128, [(3 * QTR, 512)]))
    runs.append((15 * 128, [(3 * QTR + 512, 512)]))
    return runs


def _build_nc():
    import concourse.bacc as bacc
    import concourse.tile as tile
    import concourse.mybir as mybir

    f32 = mybir.dt.float32
    f16 = mybir.dt.float16
    f8 = mybir.dt.float8e4
    Alu = mybir.AluOpType
    Act = mybir.ActivationFunctionType

    nc = bacc.Bacc("TRN2", target_bir_lowering=False, debug=False)

    x_d = nc.dram_tensor("xi", [128, KT * N], f16, kind="ExternalInput")
    thr_d = nc.dram_tensor("thr", [128, 1], f32, kind="ExternalInput")
    thb_d = nc.dram_tensor("thb", [128, 1], f32, kind="ExternalInput")
    mk_d = nc.dram_tensor("mk", [HALF, N], f8, kind="ExternalOutput")

    with tile.TileContext(nc) as tc:
        with tc.tile_pool(name="const", bufs=1) as cpool, \
             tc.tile_pool(name="psum", bufs=8, space="PSUM") as psum, \
             tc.tile_pool(name="mk", bufs=8) as mkp:
            xt = cpool.tile([128, KT * N], f16, name="xt")
            # stage 1: own-half cols right-to-left; each chunk unlocks
            # several times more PE work than the next chunk's load time,
            # so the PE never stalls (a stall resets the max p-state)
            for lo, hi in ((1024, 2048), (512, 1024), (0, 512)):
                for k in range(KT):
                    c0 = k * N
                    nc.sync.dma_start(
                        out=xt[:, c0 + lo:c0 + hi],
                        in_=x_d.ap()[:, c0 + lo:c0 + hi])
            thr_t = cpool.tile([128, 1], f32, name="thr_t")
            nc.sync.dma_start(out=thr_t[:], in_=thr_d.ap())
            thb_t = cpool.tile([128, 1], f32, name="thb_t")
            nc.sync.dma_start(out=thb_t[:], in_=thb_d.ap())
            # stage 2: cross cols
            for k in range(KT):
                c0 = k * N + HALF
                nc.sync.dma_start(
                    out=xt[:, c0:c0 + HALF], in_=x_d.ap()[:, c0:c0 + HALF])

            act_w = 0.0
            dve_w = 0.0
            for i0, pieces in _runs():
                jbase = pieces[0][0]
                w_all = pieces[-1][0] + pieces[-1][1] - jbase
                yr = mkp.tile([128, 2048], f8, name="yr")
                for j0, w in pieces:
                    ps = psum.tile([128, 512], f32, name="ps")
                    for k in range(KT):
                        nc.tensor.matmul(
                            ps[:, :w],
                            xt[:, k * N + i0:k * N + i0 + 128],
                            xt[:, k * N + j0:k * N + j0 + w],
                            start=(k == 0), stop=(k == KT - 1),
                        )
                    ysl = yr[:, j0 - jbase:j0 - jbase + w]
                    if act_w / 1.2 <= dve_w / 0.96:
                        nc.scalar.activation(
                            ysl, ps[:, :w], Act.Identity,
                            bias=thb_t[:], scale=YSCALE)
                        act_w += w
                    else:
                        nc.vector.tensor_scalar(
                            ysl, ps[:, :w], thr_t[:], YSCALE,
                            op0=Alu.subtract, op1=Alu.mult)
                        dve_w += w
                nc.sync.dma_start(
                    out=mk_d.ap()[i0:i0 + 128, jbase:jbase + w_all],
                    in_=yr[:, :w_all])
    nc.compile()
    return nc


def get_nc():
    global _compiled_nc
    if _compiled_nc is None:
        _compiled_nc = _build_nc()
    return _compiled_nc


def make_inputs(x):
    xs = np.asarray(x)[:, :, :, 0]                      # (B, C, N) fp32
    nrm = np.sqrt(np.sum(xs * xs, axis=1, keepdims=True))
    xn = xs / np.maximum(nrm, EPS)

    Nsq = float(N) * float(N)
    in_maps = []
    xn64_all, tg_all = [], []
    for b in range(B):
        xb64 = xn[b].astype(np.float64)                 # (C, N)
        s = xb64.sum(axis=1)
        Cm = xb64 @ xb64.T
        sum_g = float(s @ s)
        sum_g2 = float((Cm * Cm).sum())
        mean = (2.0 * sum_g - 2.0 * Nsq) / Nsq
        s2 = 4.0 * sum_g2 - 8.0 * sum_g + 4.0 * Nsq
        var = (s2 - Nsq * mean * mean) / (Nsq - 1.0)
        t_g = (mean + PPF_09 * np.sqrt(var) + 2.0) / 2.0

        xn64_all.append(xb64)
        tg_all.append(float(t_g))
        thr_dev = np.full((128, 1), t_g, np.float32)
        thb_dev = np.full((128, 1), -t_g * YSCALE, np.float32)

        for h in range(2):
            own = xn[b][:, h * HALF:(h + 1) * HALF]
            if h == 0:
                cross = xn[b][:, HALF:]                  # Q2, Q3
            else:
                cross = np.concatenate(
                    [xn[b][:, QTR:HALF], xn[b][:, :QTR]], axis=1)
            xloc = np.concatenate([own, cross], axis=1)  # (C, N)
            xkt = xloc.reshape(KT, 128, N).transpose(1, 0, 2)
            in_maps.append({
                "xi": np.ascontiguousarray(
                    xkt.reshape(128, KT * N).astype(np.float16)),
                "thr": thr_dev,
                "thb": thb_dev,
            })
    _last_ctx["xn64"] = xn64_all
    _last_ctx["tg"] = tg_all
    return in_maps


def assemble(results):
    import ml_dtypes

    out = np.empty((2, B * N * N), np.int32)
    for b in range(B):
        y_full = np.empty((N, N), np.float32)
        for h in range(2):
            raw = np.asarray(results[2 * b + h]["mk"]).reshape(HALF, N)
            y = raw.view(ml_dtypes.float8_e4m3).astype(np.float32)
            # diag block: computed exactly on the upper wedge
            d = y[:, :HALF]
            d = np.triu(d) + np.triu(d, 1).T
            r0 = h * HALF
            y_full[r0:r0 + HALF, r0:r0 + HALF] = d
            if h == 0:
                y_full[0:QTR, HALF:3 * QTR] = y[0:QTR, HALF:3 * QTR]
                y_full[QTR:HALF, 3 * QTR:N] = y[QTR:HALF, 3 * QTR:N]
            else:
                y_full[HALF:3 * QTR, QTR:HALF] = y[0:QTR, HALF:3 * QTR]
                y_full[3 * QTR:N, 0:QTR] = y[QTR:HALF, 3 * QTR:N]
        y_full[HALF:3 * QTR, 0:QTR] = y_full[0:QTR, HALF:3 * QTR].T
        y_full[3 * QTR:N, QTR:HALF] = y_full[QTR:HALF, 3 * QTR:N].T
        y_full[QTR:HALF, HALF:3 * QTR] = y_full[HALF:3 * QTR, QTR:HALF].T
        y_full[0:QTR, 3 * QTR:N] = y_full[3 * QTR:N, 0:QTR].T

        m = y_full > 0.0
        ii, jj = np.nonzero(np.abs(y_full) <= BAND)
        if ii.size:
            xb = _last_ctx["xn64"][b]
            tg = _last_ctx["tg"][b]
            ge = np.einsum('ck,ck->k', xb[:, ii], xb[:, jj])
            m[ii, jj] = ge > tg
        rows = (b * N + np.arange(N, dtype=np.int32))
        base = b * N * N
        e0 = out[0, base:base + N * N].reshape(N, N)
        e1 = out[1, base:base + N * N].reshape(N, N)
        e0.fill(-1)
        e1.fill(-1)
        np.copyto(e0, np.broadcast_to(rows[:, None], (N, N)), where=m)
        np.copyto(e1, np.broadcast_to(rows[None, :], (N, N)), where=m)
    return out


def kernel(x):
    from concourse.bass_utils import run_bass_kernel_spmd

    nc = get_nc()
    in_maps = make_inputs(x)
    res = run_bass_kernel_spmd(nc, in_maps, list(range(NCORES)))
    return assemble(res.results)


# revision 3
# speedup vs baseline: 1.3082x; 1.3082x over previous
"""DenseDilatedKnnGraph edge extraction on 8 NeuronCores.

Per batch element (4 total), the adjacency is a thresholded gram matrix
of the channel-normalized points: G = x_n^T x_n (4096x4096, C=384),
edge iff standardized similarity > ppf(0.9). Mean/std reduce to
sum(G) = |sum_i x_i|^2 and sum(G^2) = ||X^T X||_F^2 (a 384x384 gram),
so the host computes the per-batch threshold t_g exactly in f64 at
negligible cost, and the device only computes G tiles and compares.

Device kernel (SPMD, identical NEFF on all 8 cores, 2 cores per batch):
- x is fp16 (11 significant bits, measured equal to what the PE's
  fp32r mode keeps internally, at half the input DMA). One fp16 matmul
  per 128-channel block, [128 x <=512] tiles, fp32 PSUM accumulation.
- Unique-tile partition: each core owns half the rows; it computes its
  own diagonal block's upper wedge exactly (row block rb covers cols
  [rb*128, 2048), pieces of 512/384/256/128) and exactly half of the
  A x B cross block, split quarter-wise so the two cores of a batch are
  perfectly disjoint: core0 Q0xQ2 + Q1xQ3, core1 Q2xQ1 + Q3xQ0. No
  gram entry is computed twice anywhere (fp16 products + fixed
  accumulation order make G bitwise symmetric, so transposed copies
  are exact).
- Output is y = (G - t_g) * 256 in fp8 (1 byte/entry, ~4.4MB/core vs
  64MB of int32 edge indices): sign gives the adjacency; host resolves
  the narrow band |y| <= BAND with exact f64 dot products (~20k pairs
  per batch) and reconstructs int32 edge indices with np.where.
- Schedule: input streams in right-to-left 512-col chunks sized so the
  PE starts after ~0.5MB and never stalls (a stall resets the PE's max
  p-state); diagonal row blocks run descending, then cross blocks.
  y ops alternate between the Activation engine (Identity with
  per-partition bias) and DVE, balanced by column count; each row
  block's y leaves in one batched DMA (up to 2KB lines). Input DMAs
  issue from the Act HWDGE ring, output from the SP ring.

Measured: ~61.1us HW exec (baseline 246.7us), rel err 1.24e-3
(24/134M boundary flips from f64-vs-f32 threshold stats, same as the
fp16 hi/lo baseline).
"""

import sys

for _p in ("/opt/trn_rl_repo", "/root/.axon_site/_ro/trn_rl_repo"):
    if _p not in sys.path:
        sys.path.append(_p)

import numpy as np

B, C, N = 4, 384, 4096
HALF = N // 2
QTR = N // 4
KT = C // 128
NCORES = 2 * B
PPF_09 = 1.2815515655446004
EPS = 1e-12
RB = HALF // 128
YSCALE = 256.0
BAND = 0.05

_compiled_nc = None
_last_ctx = {}


def _diag_pieces(rb):
    """Split cols [rb*128, 2048) into moving pieces, all >=256 wide
    except an unavoidable final 128 tip."""
    j, end = rb * 128, HALF
    out = []
    while j < end:
        s = end - j
        if s == 128:
            out.append((j, 128))
            j = end
        elif s == 640:
            out.extend([(j, 384), (j + 384, 256)])
            j = end
        else:
            w = min(512, s)
            out.append((j, w))
            j += w
    return out


def _runs():
    """(i0, [(j0, w)...]) per row block: diag (rb descending), cross."""
    runs = []
    for rb in range(RB - 1, -1, -1):
        runs.append((rb * 128, _diag_pieces(rb)))
    for rb in range(8):
        runs.append((rb * 128, [(HALF, 512), (HALF + 512, 512)]))
    for rb in range(8, 15):
        runs.append((rb * 128, [(3 * QTR, 512), (3 * QTR + 512, 512)]))
    runs.append((15 * 128, [(3 * QTR, 512)]))
    runs.append((15 * 128, [(3 * QTR + 512, 512)]))
    return runs


def _build_nc():
    import concourse.bacc as bacc
    import concourse.tile as tile
    import concourse.mybir as mybir

    f32 = mybir.dt.float32
    f16 = mybir.dt.float16
    f8 = mybir.dt.float8e4
    Alu = mybir.AluOpType
    Act = mybir.ActivationFunctionType

    nc = bacc.Bacc("TRN2", target_bir_lowering=False, debug=False)

    x_d = nc.dram_tensor("xi", [128, KT * N], f16, kind="ExternalInput")
    thr_d = nc.dram_tensor("thr", [128, 1], f32, kind="ExternalInput")
    thb_d = nc.dram_tensor("thb", [128, 1], f32, kind="ExternalInput")
    mk_d = nc.dram_tensor("mk", [HALF, N], f8, kind="ExternalOutput")

    with tile.TileContext(nc) as tc:
        with tc.tile_pool(name="const", bufs=1) as cpool, \
             tc.tile_pool(name="psum", bufs=8, space="PSUM") as psum, \
             tc.tile_pool(name="mk", bufs=8) as mkp:
            xt = cpool.tile([128, KT * N], f16, name="xt")
            # stage 1: own-half cols right-to-left; each chunk unlocks
            # several times more PE work than the next chunk's load time,
            # so the PE never stalls (a stall resets the max p-state)
            for lo, hi in ((1280, 2048), (640, 1280), (0, 640)):
                for k in range(KT):
                    c0 = k * N
                    nc.scalar.dma_start(
                        out=xt[:, c0 + lo:c0 + hi],
                        in_=x_d.ap()[:, c0 + lo:c0 + hi])
            thr_t = cpool.tile([128, 1], f32, name="thr_t")
            nc.sync.dma_start(out=thr_t[:], in_=thr_d.ap())
            thb_t = cpool.tile([128, 1], f32, name="thb_t")
            nc.sync.dma_start(out=thb_t[:], in_=thb_d.ap())
            # stage 2: cross cols
            for k in range(KT):
                c0 = k * N + HALF
                nc.scalar.dma_start(
                    out=xt[:, c0:c0 + HALF], in_=x_d.ap()[:, c0:c0 + HALF])

            act_w = 0.0
            dve_w = 0.0
            for i0, pieces in _runs():
                jbase = pieces[0][0]
                w_all = pieces[-1][0] + pieces[-1][1] - jbase
                yr = mkp.tile([128, 2048], f8, name="yr")
                for j0, w in pieces:
                    ps = psum.tile([128, 512], f32, name="ps")
                    for k in range(KT):
                        nc.tensor.matmul(
                            ps[:, :w],
                            xt[:, k * N + i0:k * N + i0 + 128],
                            xt[:, k * N + j0:k * N + j0 + w],
                            start=(k == 0), stop=(k == KT - 1),
                        )
                    ysl = yr[:, j0 - jbase:j0 - jbase + w]
                    if act_w / 1.2 <= dve_w / 0.96:
                        nc.scalar.activation(
                            ysl, ps[:, :w], Act.Identity,
                            bias=thb_t[:], scale=YSCALE)
                        act_w += w
                    else:
                        nc.vector.tensor_scalar(
                            ysl, ps[:, :w], thr_t[:], YSCALE,
                            op0=Alu.subtract, op1=Alu.mult)
                        dve_w += w
                nc.sync.dma_start(
                    out=mk_d.ap()[i0:i0 + 128, jbase:jbase + w_all],
                    in_=yr[:, :w_all])
    nc.compile()
    return nc


def get_nc():
    global _compiled_nc
    if _compiled_nc is None:
        _compiled_nc = _build_nc()
    return _compiled_nc


def make_inputs(x):
    xs = np.asarray(x)[:, :, :, 0]                      # (B, C, N) fp32
    nrm = np.sqrt(np.sum(xs * xs, axis=1, keepdims=True))
    xn = xs / np.maximum(nrm, EPS)

    Nsq = float(N) * float(N)
    in_maps = []
    xn64_all, tg_all = [], []
    for b in range(B):
        xb64 = xn[b].astype(np.float64)                 # (C, N)
        s = xb64.sum(axis=1)
        Cm = xb64 @ xb64.T
        sum_g = float(s @ s)
        sum_g2 = float((Cm * Cm).sum())
        mean = (2.0 * sum_g - 2.0 * Nsq) / Nsq
        s2 = 4.0 * sum_g2 - 8.0 * sum_g + 4.0 * Nsq
        var = (s2 - Nsq * mean * mean) / (Nsq - 1.0)
        t_g = (mean + PPF_09 * np.sqrt(var) + 2.0) / 2.0

        xn64_all.append(xb64)
        tg_all.append(float(t_g))
        thr_dev = np.full((128, 1), t_g, np.float32)
        thb_dev = np.full((128, 1), -t_g * YSCALE, np.float32)

        for h in range(2):
            own = xn[b][:, h * HALF:(h + 1) * HALF]
            if h == 0:
                cross = xn[b][:, HALF:]                  # Q2, Q3
            else:
                cross = np.concatenate(
                    [xn[b][:, QTR:HALF], xn[b][:, :QTR]], axis=1)
            xloc = np.concatenate([own, cross], axis=1)  # (C, N)
            xkt = xloc.reshape(KT, 128, N).transpose(1, 0, 2)
            in_maps.append({
                "xi": np.ascontiguousarray(
                    xkt.reshape(128, KT * N).astype(np.float16)),
                "thr": thr_dev,
                "thb": thb_dev,
            })
    _last_ctx["xn64"] = xn64_all
    _last_ctx["tg"] = tg_all
    return in_maps


def assemble(results):
    import ml_dtypes

    out = np.empty((2, B * N * N), np.int32)
    for b in range(B):
        y_full = np.empty((N, N), np.float32)
        for h in range(2):
            raw = np.asarray(results[2 * b + h]["mk"]).reshape(HALF, N)
            y = raw.view(ml_dtypes.float8_e4m3).astype(np.float32)
            # diag block: computed exactly on the upper wedge
            d = y[:, :HALF]
            d = np.triu(d) + np.triu(d, 1).T
            r0 = h * HALF
            y_full[r0:r0 + HALF, r0:r0 + HALF] = d
            if h == 0:
                y_full[0:QTR, HALF:3 * QTR] = y[0:QTR, HALF:3 * QTR]
                y_full[QTR:HALF, 3 * QTR:N] = y[QTR:HALF, 3 * QTR:N]
            else:
                y_full[HALF:3 * QTR, QTR:HALF] = y[0:QTR, HALF:3 * QTR]
                y_full[3 * QTR:N, 0:QTR] = y[QTR:HALF, 3 * QTR:N]
        y_full[HALF:3 * QTR, 0:QTR] = y_full[0:QTR, HALF:3 * QTR].T
        y_full[3 * QTR:N, QTR:HALF] = y_full[QTR:HALF, 3 * QTR:N].T
        y_full[QTR:HALF, HALF:3 * QTR] = y_full[HALF:3 * QTR, QTR:HALF].T
        y_full[0:QTR, 3 * QTR:N] = y_full[3 * QTR:N, 0:QTR].T

        m = y_full > 0.0
        ii, jj = np.nonzero(np.abs(y_full) <= BAND)
        if ii.size:
            xb = _last_ctx["xn64"][b]
            tg = _last_ctx["tg"][b]
            ge = np.einsum('ck,ck->k', xb[:, ii], xb[:, jj])
            m[ii, jj] = ge > tg
        rows = (b * N + np.arange(N, dtype=np.int32))
        base = b * N * N
        e0 = out[0, base:base + N * N].reshape(N, N)
        e1 = out[1, base:base + N * N].reshape(N, N)
        e0.fill(-1)
        e1.fill(-1)
        np.copyto(e0, np.broadcast_to(rows[:, None], (N, N)), where=m)
        np.copyto(e1, np.broadcast_to(rows[None, :], (N, N)), where=m)
    return out


def kernel(x):
    from concourse.bass_utils import run_bass_kernel_spmd

    nc = get_nc()
    in_maps = make_inputs(x)
    res = run_bass_kernel_spmd(nc, in_maps, list(range(NCORES)))
    return assemble(res.results)
